# revision 30
# baseline (speedup 1.0000x reference)
"""Trainium2 Bass kernel for CaMoE (LN + top-2 MoE with relu^2 FFN).

Strategy: expert-parallel over 8 NeuronCores. Core e receives only the
tokens routed to expert e (gather indices computed host-side as part of
sharding), plus W1[e]/W2[e] in bf16, pre-swizzled into per-tile lhsT
layout. On device: LayerNorm stats via ones-matmul in replicated-lane
form (stats vectors come out already broadcast across partitions),
xn = (x - mu) * rstd * sqrt(coef) in bf16, hidden = relu(xn @ W1)^2
with fp32 PSUM accumulation, y = hidden @ W2, written back
feature-major. Host scatter-adds the 8 partial outputs into x (the
residual) — pure unsharding, no collectives needed.

Self-contained: hardcodes shapes B=4, T=2048, C=1024, E=8, H=4096.
"""

import os
import sys

for _p in ("/opt/trn_rl_repo", "/root/.axon_site/_ro/trn_rl_repo"):
    if os.path.isdir(_p) and _p not in sys.path:
        sys.path.insert(0, _p)

from contextlib import ExitStack

import ml_dtypes
import numpy as np

import concourse.bass as bass
import concourse.tile as tile
from concourse import bacc, mybir
from concourse.bass_utils import run_bass_kernel_spmd

N_CORES = 8
C = 1024
H = 4096
NB = 512          # token block (matmul moving free dim)
NC_T = C // 128   # 8 c-tiles
NH_T = H // 128   # 32 h-tiles
EPS = 1e-5

F32 = mybir.dt.float32
BF16 = mybir.dt.bfloat16
AF = mybir.ActivationFunctionType
OP = mybir.AluOpType


def _build_kernel(NT: int, has_beta: bool):
    """Build the per-core SPMD program for NT padded tokens."""
    blocks = []
    t0 = 0
    while t0 < NT:
        tn = min(NB, NT - t0)
        blocks.append((t0, tn))
        t0 += tn
    nblk = len(blocks)
    nc = bacc.Bacc("TRN2", target_bir_lowering=False, debug=False, num_devices=1)

    xgt_d = nc.dram_tensor("xgt", [C, NT], F32, kind="ExternalInput").ap()
    # weights pre-swizzled on host into per-tile lhsT layout:
    #   w1[h][p, c*128+j] = (gamma*W1)[c*128+p, h*128+j]
    #   w2[c][p, h*128+j] = W2[h*128+p, c*128+j]
    w1_d = nc.dram_tensor("w1", [NH_T, 128, C], BF16, kind="ExternalInput").ap()
    w2_d = nc.dram_tensor("w2", [NC_T, 128, H], BF16, kind="ExternalInput").ap()
    cg_d = nc.dram_tensor("cg", [1, NT], F32, kind="ExternalInput").ap()
    if has_beta:
        bias1_d = nc.dram_tensor("bias1", [128, NH_T], F32, kind="ExternalInput").ap()
    ygt_d = nc.dram_tensor("ygt", [C, NT], F32, kind="ExternalOutput").ap()

    with tile.TileContext(nc) as tc, ExitStack() as ctx:
        sb = ctx.enter_context(tc.tile_pool(name="sb", bufs=1))
        ps = ctx.enter_context(tc.tile_pool(name="ps", bufs=1, space="PSUM"))

        # ---- constants ----
        ones_k = sb.tile([128, 128], BF16, tag="ones_k", bufs=1)
        nc.vector.memset(ones_k, 1.0)
        eps_t = sb.tile([128, 1], F32, tag="eps", bufs=1)
        nc.vector.memset(eps_t, EPS)
        warm_t = sb.tile([128, 1], F32, tag="warm", bufs=1)
        nc.scalar.activation(warm_t, eps_t, AF.Square)
        if has_beta:
            b1sb = sb.tile([128, NH_T], F32, tag="b1", bufs=1)
            nc.sync.dma_start(b1sb, bias1_d)

        def stats_phase(blk):
            """LN stats for block blk, replicated-lane form.

            Returns [128,tn] scale/shift (already broadcast across
            partitions) plus the raw x tiles (kept for normalize)."""
            t0, tn = blocks[blk]
            tsl = bass.ds(t0, tn)
            sum_ps = ps.tile([128, tn], F32, tag="stat", bufs=3, name=f"sum{blk}")
            sq_ps = ps.tile([128, tn], F32, tag="stat", bufs=3, name=f"sq{blk}")
            xs = []
            eng = [nc.sync, nc.scalar, nc.sync, nc.scalar,
                   nc.sync, nc.scalar, nc.sync, nc.scalar]
            for c in range(NC_T):
                xt = sb.tile([128, tn], F32, tag="xs", bufs=14, name=f"xa{blk}_{c}", padded_shape=[128, NB])
                q = eng[c] if blk == 0 else nc.sync
                q.dma_start(xt, xgt_d[c * 128:(c + 1) * 128, tsl])
                xb = sb.tile([128, tn], BF16, tag="xb16", bufs=3, name=f"xb16{blk}_{c}", padded_shape=[128, NB])
                nc.vector.tensor_copy(xb, xt)
                nc.tensor.matmul(sum_ps, ones_k, xb,
                                 start=(c == 0), stop=(c == NC_T - 1))
                xs.append(xt)
            for c in range(NC_T):
                xsq = sb.tile([128, tn], BF16, tag="xsq", bufs=3, name=f"xsq{blk}_{c}", padded_shape=[128, NB])
                nc.scalar.activation(xsq, xs[c], AF.Square)
                nc.tensor.matmul(sq_ps, ones_k, xsq,
                                 start=(c == 0), stop=(c == NC_T - 1))
            vmu = sb.tile([128, tn], F32, tag="vec", bufs=3, name=f"vmu{blk}", padded_shape=[128, NB])
            nc.vector.tensor_scalar_mul(vmu, sum_ps, 1.0 / C)
            # var = sq/C - mu^2
            vvar = sb.tile([128, tn], F32, tag="vec", bufs=3, name=f"vvar{blk}", padded_shape=[128, NB])
            nc.vector.scalar_tensor_tensor(vvar, vmu, -1.0, vmu, OP.mult, OP.mult)
            nc.vector.scalar_tensor_tensor(vvar, sq_ps, 1.0 / C, vvar, OP.mult, OP.add)
            vstd = sb.tile([128, tn], F32, tag="vec", bufs=3, name=f"vstd{blk}", padded_shape=[128, NB])
            nc.scalar.activation(vstd, vvar, AF.Sqrt, bias=eps_t)
            vrstd = sb.tile([128, tn], F32, tag="vec", bufs=3, name=f"vrstd{blk}", padded_shape=[128, NB])
            nc.vector.reciprocal_approx_fast(out=vrstd, in_=vstd)
            vcg = sb.tile([128, tn], F32, tag="bc", bufs=8, name=f"vcg{blk}", padded_shape=[128, NB])
            nc.sync.dma_start(vcg, cg_d[0:1, tsl].to_broadcast([128, tn]))
            if has_beta:
                vs = vrstd          # coef applied on the output instead
            else:
                vs = sb.tile([128, tn], F32, tag="bc", bufs=8, name=f"vs{blk}", padded_shape=[128, NB])
                nc.vector.tensor_mul(vs, vrstd, vcg)
            vb = sb.tile([128, tn], F32, tag="bc", bufs=8, name=f"vb{blk}", padded_shape=[128, NB])
            nc.vector.scalar_tensor_tensor(vb, vmu, -1.0, vs, OP.mult, OP.mult)
            return vs, vb, vcg, xs

        def normalize_phase(blk, vs, vb, xs):
            t0, tn = blocks[blk]
            xn = []
            for c in range(NC_T):
                xt = xs[c]
                nc.vector.tensor_mul(xt, xt, vs)
                xnc = sb.tile([128, tn], BF16, tag="xn", bufs=20, name=f"xn{blk}_{c}", padded_shape=[128, NB])
                nc.vector.tensor_add(xnc, xt, vb)
                xn.append(xnc)
            return xn

        def mm1_phase(blk, xn, mid_hook=None):
            t0, tn = blocks[blk]
            hid = []
            w2ts = []
            for h in range(NH_T):
                if h == 16 and mid_hook is not None:
                    mid_hook()
                if h >= 8 and h % 3 == 2 and len(w2ts) < NC_T:
                    c = len(w2ts)
                    w2t = sb.tile([128, H], BF16, tag="w2s", bufs=6,
                                  name=f"w2t{blk}_{c}")
                    nc.scalar.dma_start(w2t, w2_d[c])
                    w2ts.append(w2t)
                w1t = sb.tile([128, C], BF16, tag="w1s", bufs=8, name=f"w1t{blk}_{h}")
                nc.gpsimd.dma_start(w1t, w1_d[h])
                pa = ps.tile([128, tn], F32, tag="mm", bufs=4, name=f"pa{blk}_{h}")
                for c in range(NC_T):
                    nc.tensor.matmul(pa, w1t[:, c * 128:(c + 1) * 128], xn[c],
                                     start=(c == 0), stop=(c == NC_T - 1))
                if has_beta:
                    nc.vector.tensor_scalar_add(pa, pa, b1sb[:, h:h + 1])
                # relu(x)^2 == max(x,0)*x; DVE may read only one PSUM operand
                rt = sb.tile([128, tn], BF16, tag="rt", bufs=3, name=f"r{blk}_{h}", padded_shape=[128, NB])
                nc.vector.tensor_scalar_max(rt, pa, 0.0)
                ht = sb.tile([128, tn], BF16, tag="hid", bufs=44, name=f"h{blk}_{h}", padded_shape=[128, NB])
                nc.vector.tensor_mul(ht, rt, pa)
                hid.append(ht)
            while len(w2ts) < NC_T:
                c = len(w2ts)
                w2t = sb.tile([128, H], BF16, tag="w2s", bufs=6,
                              name=f"w2t{blk}_{c}")
                nc.scalar.dma_start(w2t, w2_d[c])
                w2ts.append(w2t)
            return hid, w2ts

        def mm2_phase(blk, hid, w2ts, vcf):
            t0, tn = blocks[blk]
            tsl = bass.ds(t0, tn)
            for c in range(NC_T):
                w2t = w2ts[c]
                pb = ps.tile([128, tn], F32, tag="mm", bufs=4, name=f"pb{blk}_{c}")
                for h in range(NH_T):
                    nc.tensor.matmul(pb, w2t[:, h * 128:(h + 1) * 128], hid[h],
                                     start=(h == 0), stop=(h == NH_T - 1))
                ot = sb.tile([128, tn], F32, tag="out", bufs=4, name=f"o{blk}_{c}", padded_shape=[128, NB])
                if has_beta:
                    nc.vector.tensor_mul(ot, pb, vcf)
                else:
                    nc.vector.tensor_copy(ot, pb)
                nc.sync.dma_start(ygt_d[c * 128:(c + 1) * 128, tsl], ot)

        # Software pipeline: stats/normalize of blk+1 are emitted so the PE
        # runs them inside blk's mm1/mm2 stream with no gaps.
        vs0, vb0, vcf, xs0 = stats_phase(0)
        xn = normalize_phase(0, vs0, vb0, xs0)
        nxt = {}
        for blk in range(nblk):
            def mid_hook(b=blk):
                nxt.update(zip(("vs", "vb", "vcf", "xs"), stats_phase(b + 1)))
            hid, w2ts = mm1_phase(blk, xn, mid_hook if blk + 1 < nblk else None)
            if blk + 1 < nblk:
                xn = normalize_phase(blk + 1, nxt["vs"], nxt["vb"], nxt["xs"])
            mm2_phase(blk, hid, w2ts, vcf)
            if blk + 1 < nblk:
                vcf = nxt["vcf"]

    nc.compile()
    return nc


_KERNEL_CACHE = {}


def _get_kernel(NT: int, has_beta: bool):
    key = (NT, has_beta)
    if key not in _KERNEL_CACHE:
        _KERNEL_CACHE[key] = _build_kernel(NT, has_beta)
    return _KERNEL_CACHE[key]


def kernel(x, weights, gamma, beta, W1, W2, winners):
    x = np.asarray(x, dtype=np.float32)
    weights = np.asarray(weights, dtype=np.float32)
    gamma = np.asarray(gamma, dtype=np.float32)
    beta = np.asarray(beta, dtype=np.float32)
    W1 = np.asarray(W1, dtype=np.float32)
    W2 = np.asarray(W2, dtype=np.float32)
    winners = np.asarray(winners)

    B, T, C_ = x.shape
    E = W1.shape[0]
    assert C_ == C and E == N_CORES and W1.shape[2] == H

    x_flat = x.reshape(-1, C)
    win = winners.reshape(-1, 2)
    wts = weights.reshape(-1, 2)

    has_beta = bool(np.any(beta != 0.0))

    # ---- host-side routing (sharding prep) ----
    idxs, coefs = [], []
    for e in range(E):
        m = win == e
        tok = np.nonzero(m.any(axis=1))[0]
        cf = (wts * m).sum(axis=1)[tok]
        idxs.append(tok)
        coefs.append(cf.astype(np.float32))
    NT = int(np.ceil(max(len(t) for t in idxs) / 8) * 8)

    in_maps = []
    for e in range(E):
        tok, cf = idxs[e], coefs[e]
        n = len(tok)
        xg = np.zeros((NT, C), np.float32)
        xg[:n] = x_flat[tok]
        cg = np.zeros((1, NT), np.float32)
        # no beta: fold sqrt(coef) into the LN scale (relu^2 is 2-homogeneous
        # and W2 linear, so scaling xn by sqrt(c) scales the output by c).
        cg[0, :n] = cf if has_beta else np.sqrt(cf)
        w1g = (W1[e] * gamma[:, None]).astype(ml_dtypes.bfloat16)
        w1r = np.ascontiguousarray(
            w1g.reshape(NC_T, 128, NH_T, 128).transpose(2, 1, 0, 3)
        ).reshape(NH_T, 128, C)
        w2r = np.ascontiguousarray(
            W2[e].astype(ml_dtypes.bfloat16)
            .reshape(NH_T, 128, NC_T, 128).transpose(2, 1, 0, 3)
        ).reshape(NC_T, 128, H)
        m = {
            "xgt": np.ascontiguousarray(xg.T),
            "w1": w1r,
            "w2": w2r,
            "cg": cg,
        }
        if has_beta:
            b1 = (beta @ W1[e]).astype(np.float32)          # [H]
            m["bias1"] = np.ascontiguousarray(b1.reshape(NH_T, 128).T)
        in_maps.append(m)

    nc = _get_kernel(NT, has_beta)
    res = run_bass_kernel_spmd(nc, in_maps, list(range(N_CORES)))

    # ---- host-side unshard: scatter-add partial expert outputs ----
    out = x_flat.copy()
    for e in range(E):
        yg = res.results[e]["ygt"]                          # [C, NT]
        n = len(idxs[e])
        out[idxs[e]] += yg.T[:n]
    return out.reshape(B, T, C).astype(np.float32)


# revision 31
# speedup vs baseline: 1.0105x; 1.0105x over previous
"""Trainium2 Bass kernel for CaMoE (LN + top-2 MoE with relu^2 FFN).

Strategy: expert-parallel over 8 NeuronCores. Core e receives only the
tokens routed to expert e (gather indices computed host-side as part of
sharding), plus W1[e]/W2[e] in bf16, pre-swizzled into per-tile lhsT
layout. On device: LayerNorm stats via ones-matmul in replicated-lane
form (stats vectors come out already broadcast across partitions),
xn = (x - mu) * rstd * sqrt(coef) in bf16, hidden = relu(xn @ W1)^2
with fp32 PSUM accumulation, y = hidden @ W2, written back
feature-major. Host scatter-adds the 8 partial outputs into x (the
residual) — pure unsharding, no collectives needed.

Self-contained: hardcodes shapes B=4, T=2048, C=1024, E=8, H=4096.
"""

import os
import sys

for _p in ("/opt/trn_rl_repo", "/root/.axon_site/_ro/trn_rl_repo"):
    if os.path.isdir(_p) and _p not in sys.path:
        sys.path.insert(0, _p)

from contextlib import ExitStack

import ml_dtypes
import numpy as np

import concourse.bass as bass
import concourse.tile as tile
from concourse import bacc, mybir
from concourse.bass_utils import run_bass_kernel_spmd

N_CORES = 8
C = 1024
H = 4096
NB = 512          # token block (matmul moving free dim)
NC_T = C // 128   # 8 c-tiles
NH_T = H // 128   # 32 h-tiles
EPS = 1e-5

F32 = mybir.dt.float32
BF16 = mybir.dt.bfloat16
AF = mybir.ActivationFunctionType
OP = mybir.AluOpType


def _build_kernel(NT: int, has_beta: bool):
    """Build the per-core SPMD program for NT padded tokens."""
    blocks = []
    t0 = 0
    while t0 < NT:
        tn = min(NB, NT - t0)
        blocks.append((t0, tn))
        t0 += tn
    nblk = len(blocks)
    nc = bacc.Bacc("TRN2", target_bir_lowering=False, debug=False, num_devices=1)

    xgt_d = nc.dram_tensor("xgt", [C, NT], F32, kind="ExternalInput").ap()
    # weights pre-swizzled on host into per-tile lhsT layout:
    #   w1[h][p, c*128+j] = (gamma*W1)[c*128+p, h*128+j]
    #   w2[c][p, h*128+j] = W2[h*128+p, c*128+j]
    w1_d = nc.dram_tensor("w1", [NH_T, 128, C], BF16, kind="ExternalInput").ap()
    w2_d = nc.dram_tensor("w2", [NC_T, 128, H], BF16, kind="ExternalInput").ap()
    cg_d = nc.dram_tensor("cg", [1, NT], F32, kind="ExternalInput").ap()
    if has_beta:
        bias1_d = nc.dram_tensor("bias1", [128, NH_T], F32, kind="ExternalInput").ap()
    ygt_d = nc.dram_tensor("ygt", [C, NT], F32, kind="ExternalOutput").ap()

    with tile.TileContext(nc) as tc, ExitStack() as ctx:
        sb = ctx.enter_context(tc.tile_pool(name="sb", bufs=1))
        ps = ctx.enter_context(tc.tile_pool(name="ps", bufs=1, space="PSUM"))

        # ---- constants ----
        ones_k = sb.tile([128, 128], BF16, tag="ones_k", bufs=1)
        nc.vector.memset(ones_k, 1.0)
        eps_t = sb.tile([128, 1], F32, tag="eps", bufs=1)
        nc.vector.memset(eps_t, EPS)
        warm_t = sb.tile([128, 1], F32, tag="warm", bufs=1)
        nc.scalar.activation(warm_t, eps_t, AF.Square)
        if has_beta:
            b1sb = sb.tile([128, NH_T], F32, tag="b1", bufs=1)
            nc.sync.dma_start(b1sb, bias1_d)

        def stats_phase(blk):
            """LN stats for block blk, replicated-lane form.

            Returns [128,tn] scale/shift (already broadcast across
            partitions) plus the raw x tiles (kept for normalize)."""
            t0, tn = blocks[blk]
            tsl = bass.ds(t0, tn)
            sum_ps = ps.tile([128, tn], F32, tag="stat", bufs=3, name=f"sum{blk}")
            sq_ps = ps.tile([128, tn], F32, tag="stat", bufs=3, name=f"sq{blk}")
            xs = []
            for c in range(NC_T):
                xt = sb.tile([128, tn], F32, tag="xs", bufs=14, name=f"xa{blk}_{c}", padded_shape=[128, NB])
                nc.sync.dma_start(xt, xgt_d[c * 128:(c + 1) * 128, tsl])
                xb = sb.tile([128, tn], BF16, tag="xb16", bufs=3, name=f"xb16{blk}_{c}", padded_shape=[128, NB])
                nc.vector.tensor_copy(xb, xt)
                xsq = sb.tile([128, tn], BF16, tag="xsq", bufs=3, name=f"xsq{blk}_{c}", padded_shape=[128, NB])
                nc.scalar.activation(xsq, xt, AF.Square)
                nc.tensor.matmul(sum_ps, ones_k, xb,
                                 start=(c == 0), stop=(c == NC_T - 1))
                nc.tensor.matmul(sq_ps, ones_k, xsq,
                                 start=(c == 0), stop=(c == NC_T - 1))
                xs.append(xt)
            vmu = sb.tile([128, tn], F32, tag="vec", bufs=3, name=f"vmu{blk}", padded_shape=[128, NB])
            nc.vector.tensor_scalar_mul(vmu, sum_ps, 1.0 / C)
            # var = sq/C - mu^2
            vvar = sb.tile([128, tn], F32, tag="vec", bufs=3, name=f"vvar{blk}", padded_shape=[128, NB])
            nc.vector.scalar_tensor_tensor(vvar, vmu, -1.0, vmu, OP.mult, OP.mult)
            nc.vector.scalar_tensor_tensor(vvar, sq_ps, 1.0 / C, vvar, OP.mult, OP.add)
            vstd = sb.tile([128, tn], F32, tag="vec", bufs=3, name=f"vstd{blk}", padded_shape=[128, NB])
            nc.scalar.activation(vstd, vvar, AF.Sqrt, bias=eps_t)
            vrstd = sb.tile([128, tn], F32, tag="vec", bufs=3, name=f"vrstd{blk}", padded_shape=[128, NB])
            nc.vector.reciprocal_approx_fast(out=vrstd, in_=vstd)
            vcg = sb.tile([128, tn], F32, tag="bc", bufs=8, name=f"vcg{blk}", padded_shape=[128, NB])
            nc.sync.dma_start(vcg, cg_d[0:1, tsl].to_broadcast([128, tn]))
            if has_beta:
                vs = vrstd          # coef applied on the output instead
            else:
                vs = sb.tile([128, tn], F32, tag="bc", bufs=8, name=f"vs{blk}", padded_shape=[128, NB])
                nc.vector.tensor_mul(vs, vrstd, vcg)
            vb = sb.tile([128, tn], F32, tag="bc", bufs=8, name=f"vb{blk}", padded_shape=[128, NB])
            nc.vector.scalar_tensor_tensor(vb, vmu, -1.0, vs, OP.mult, OP.mult)
            return vs, vb, vcg, xs

        def normalize_phase(blk, vs, vb, xs):
            t0, tn = blocks[blk]
            xn = []
            for c in range(NC_T):
                xt = xs[c]
                nc.vector.tensor_mul(xt, xt, vs)
                xnc = sb.tile([128, tn], BF16, tag="xn", bufs=20, name=f"xn{blk}_{c}", padded_shape=[128, NB])
                nc.vector.tensor_add(xnc, xt, vb)
                xn.append(xnc)
            return xn

        def mm1_phase(blk, xn, mid_hook=None):
            t0, tn = blocks[blk]
            hid = []
            w2ts = []
            for h in range(NH_T):
                if h == 16 and mid_hook is not None:
                    mid_hook()
                if h >= 8 and h % 3 == 2 and len(w2ts) < NC_T:
                    c = len(w2ts)
                    w2t = sb.tile([128, H], BF16, tag="w2s", bufs=6,
                                  name=f"w2t{blk}_{c}")
                    nc.scalar.dma_start(w2t, w2_d[c])
                    w2ts.append(w2t)
                w1t = sb.tile([128, C], BF16, tag="w1s", bufs=8, name=f"w1t{blk}_{h}")
                nc.gpsimd.dma_start(w1t, w1_d[h])
                pa = ps.tile([128, tn], F32, tag="mm", bufs=4, name=f"pa{blk}_{h}")
                for c in range(NC_T):
                    nc.tensor.matmul(pa, w1t[:, c * 128:(c + 1) * 128], xn[c],
                                     start=(c == 0), stop=(c == NC_T - 1))
                if has_beta:
                    nc.vector.tensor_scalar_add(pa, pa, b1sb[:, h:h + 1])
                # relu(x)^2 == max(x,0)*x; DVE may read only one PSUM operand
                rt = sb.tile([128, tn], BF16, tag="rt", bufs=3, name=f"r{blk}_{h}", padded_shape=[128, NB])
                nc.vector.tensor_scalar_max(rt, pa, 0.0)
                ht = sb.tile([128, tn], BF16, tag="hid", bufs=44, name=f"h{blk}_{h}", padded_shape=[128, NB])
                nc.vector.tensor_mul(ht, rt, pa)
                hid.append(ht)
            while len(w2ts) < NC_T:
                c = len(w2ts)
                w2t = sb.tile([128, H], BF16, tag="w2s", bufs=6,
                              name=f"w2t{blk}_{c}")
                nc.scalar.dma_start(w2t, w2_d[c])
                w2ts.append(w2t)
            return hid, w2ts

        def mm2_phase(blk, hid, w2ts, vcf):
            t0, tn = blocks[blk]
            tsl = bass.ds(t0, tn)
            for c in range(NC_T):
                w2t = w2ts[c]
                pb = ps.tile([128, tn], F32, tag="mm", bufs=4, name=f"pb{blk}_{c}")
                for h in range(NH_T):
                    nc.tensor.matmul(pb, w2t[:, h * 128:(h + 1) * 128], hid[h],
                                     start=(h == 0), stop=(h == NH_T - 1))
                ot = sb.tile([128, tn], F32, tag="out", bufs=4, name=f"o{blk}_{c}", padded_shape=[128, NB])
                if has_beta:
                    nc.vector.tensor_mul(ot, pb, vcf)
                else:
                    nc.vector.tensor_copy(ot, pb)
                nc.sync.dma_start(ygt_d[c * 128:(c + 1) * 128, tsl], ot)

        # Software pipeline: stats/normalize of blk+1 are emitted so the PE
        # runs them inside blk's mm1/mm2 stream with no gaps.
        vs0, vb0, vcf, xs0 = stats_phase(0)
        xn = normalize_phase(0, vs0, vb0, xs0)
        nxt = {}
        for blk in range(nblk):
            def mid_hook(b=blk):
                nxt.update(zip(("vs", "vb", "vcf", "xs"), stats_phase(b + 1)))
            hid, w2ts = mm1_phase(blk, xn, mid_hook if blk + 1 < nblk else None)
            if blk + 1 < nblk:
                xn = normalize_phase(blk + 1, nxt["vs"], nxt["vb"], nxt["xs"])
            mm2_phase(blk, hid, w2ts, vcf)
            if blk + 1 < nblk:
                vcf = nxt["vcf"]

    nc.compile()
    return nc


_KERNEL_CACHE = {}


def _get_kernel(NT: int, has_beta: bool):
    key = (NT, has_beta)
    if key not in _KERNEL_CACHE:
        _KERNEL_CACHE[key] = _build_kernel(NT, has_beta)
    return _KERNEL_CACHE[key]


def kernel(x, weights, gamma, beta, W1, W2, winners):
    x = np.asarray(x, dtype=np.float32)
    weights = np.asarray(weights, dtype=np.float32)
    gamma = np.asarray(gamma, dtype=np.float32)
    beta = np.asarray(beta, dtype=np.float32)
    W1 = np.asarray(W1, dtype=np.float32)
    W2 = np.asarray(W2, dtype=np.float32)
    winners = np.asarray(winners)

    B, T, C_ = x.shape
    E = W1.shape[0]
    assert C_ == C and E == N_CORES and W1.shape[2] == H

    x_flat = x.reshape(-1, C)
    win = winners.reshape(-1, 2)
    wts = weights.reshape(-1, 2)

    has_beta = bool(np.any(beta != 0.0))

    # ---- host-side routing (sharding prep) ----
    idxs, coefs = [], []
    for e in range(E):
        m = win == e
        tok = np.nonzero(m.any(axis=1))[0]
        cf = (wts * m).sum(axis=1)[tok]
        idxs.append(tok)
        coefs.append(cf.astype(np.float32))
    NT = int(np.ceil(max(len(t) for t in idxs) / 8) * 8)

    in_maps = []
    for e in range(E):
        tok, cf = idxs[e], coefs[e]
        n = len(tok)
        xg = np.zeros((NT, C), np.float32)
        xg[:n] = x_flat[tok]
        cg = np.zeros((1, NT), np.float32)
        # no beta: fold sqrt(coef) into the LN scale (relu^2 is 2-homogeneous
        # and W2 linear, so scaling xn by sqrt(c) scales the output by c).
        cg[0, :n] = cf if has_beta else np.sqrt(cf)
        w1g = (W1[e] * gamma[:, None]).astype(ml_dtypes.bfloat16)
        w1r = np.ascontiguousarray(
            w1g.reshape(NC_T, 128, NH_T, 128).transpose(2, 1, 0, 3)
        ).reshape(NH_T, 128, C)
        w2r = np.ascontiguousarray(
            W2[e].astype(ml_dtypes.bfloat16)
            .reshape(NH_T, 128, NC_T, 128).transpose(2, 1, 0, 3)
        ).reshape(NC_T, 128, H)
        m = {
            "xgt": np.ascontiguousarray(xg.T),
            "w1": w1r,
            "w2": w2r,
            "cg": cg,
        }
        if has_beta:
            b1 = (beta @ W1[e]).astype(np.float32)          # [H]
            m["bias1"] = np.ascontiguousarray(b1.reshape(NH_T, 128).T)
        in_maps.append(m)

    nc = _get_kernel(NT, has_beta)
    res = run_bass_kernel_spmd(nc, in_maps, list(range(N_CORES)))

    # ---- host-side unshard: scatter-add partial expert outputs ----
    out = x_flat.copy()
    for e in range(E):
        yg = res.results[e]["ygt"]                          # [C, NT]
        n = len(idxs[e])
        out[idxs[e]] += yg.T[:n]
    return out.reshape(B, T, C).astype(np.float32)


# revision 32
# speedup vs baseline: 1.0327x; 1.0220x over previous
"""Trainium2 Bass kernel for CaMoE (LN + top-2 MoE with relu^2 FFN).

Strategy: expert-parallel over 8 NeuronCores. Core e receives only the
tokens routed to expert e (gather indices computed host-side as part of
sharding), plus W1[e]/W2[e] in bf16, pre-swizzled into per-tile lhsT
layout. On device: LayerNorm stats via ones-matmul in replicated-lane
form (stats vectors come out already broadcast across partitions),
xn = (x - mu) * rstd * sqrt(coef) in bf16, hidden = relu(xn @ W1)^2
with fp32 PSUM accumulation, y = hidden @ W2, written back
feature-major. Host scatter-adds the 8 partial outputs into x (the
residual) — pure unsharding, no collectives needed.

Self-contained: hardcodes shapes B=4, T=2048, C=1024, E=8, H=4096.
"""

import os
import sys

for _p in ("/opt/trn_rl_repo", "/root/.axon_site/_ro/trn_rl_repo"):
    if os.path.isdir(_p) and _p not in sys.path:
        sys.path.insert(0, _p)

from contextlib import ExitStack

import ml_dtypes
import numpy as np

import concourse.bass as bass
import concourse.tile as tile
from concourse import bacc, mybir
from concourse.bass_utils import run_bass_kernel_spmd

N_CORES = 8
C = 1024
H = 4096
NB = 512          # token block (matmul moving free dim)
NC_T = C // 128   # 8 c-tiles
NH_T = H // 128   # 32 h-tiles
EPS = 1e-5

F32 = mybir.dt.float32
BF16 = mybir.dt.bfloat16
AF = mybir.ActivationFunctionType
OP = mybir.AluOpType


def _build_kernel(NT: int, has_beta: bool):
    """Build the per-core SPMD program for NT padded tokens."""
    blocks = []
    t0 = 0
    while t0 < NT:
        tn = min(NB, NT - t0)
        blocks.append((t0, tn))
        t0 += tn
    nblk = len(blocks)
    nc = bacc.Bacc("TRN2", target_bir_lowering=False, debug=False, num_devices=1)

    xgt_d = nc.dram_tensor("xgt", [C, NT], F32, kind="ExternalInput").ap()
    # weights pre-swizzled on host into per-tile lhsT layout:
    #   w1[h][p, c*128+j] = (gamma*W1)[c*128+p, h*128+j]
    #   w2[c][p, h*128+j] = W2[h*128+p, c*128+j]
    w1_d = nc.dram_tensor("w1", [NH_T, 128, C], BF16, kind="ExternalInput").ap()
    w2_d = nc.dram_tensor("w2", [NC_T, 128, H], BF16, kind="ExternalInput").ap()
    cg_d = nc.dram_tensor("cg", [1, NT], F32, kind="ExternalInput").ap()
    if has_beta:
        bias1_d = nc.dram_tensor("bias1", [128, NH_T], F32, kind="ExternalInput").ap()
    ygt_d = nc.dram_tensor("ygt", [C, NT], F32, kind="ExternalOutput").ap()

    with tile.TileContext(nc) as tc, ExitStack() as ctx:
        sb = ctx.enter_context(tc.tile_pool(name="sb", bufs=1))
        ps = ctx.enter_context(tc.tile_pool(name="ps", bufs=1, space="PSUM"))

        # ---- constants ----
        ones_k = sb.tile([128, 128], BF16, tag="ones_k", bufs=1)
        nc.vector.memset(ones_k, 1.0)
        eps_t = sb.tile([128, 1], F32, tag="eps", bufs=1)
        nc.vector.memset(eps_t, EPS)
        if has_beta:
            b1sb = sb.tile([128, NH_T], F32, tag="b1", bufs=1)
            nc.sync.dma_start(b1sb, bias1_d)

        def stats_phase(blk):
            """LN stats for block blk, replicated-lane form.

            Returns [128,tn] scale/shift (already broadcast across
            partitions) plus the raw x tiles (kept for normalize)."""
            t0, tn = blocks[blk]
            tsl = bass.ds(t0, tn)
            sum_ps = ps.tile([128, tn], F32, tag="stat", bufs=3, name=f"sum{blk}")
            sq_ps = ps.tile([128, tn], F32, tag="stat", bufs=3, name=f"sq{blk}")
            xs = []
            for c in range(NC_T):
                xt = sb.tile([128, tn], F32, tag="xs", bufs=14, name=f"xa{blk}_{c}", padded_shape=[128, NB])
                nc.sync.dma_start(xt, xgt_d[c * 128:(c + 1) * 128, tsl])
                xb = sb.tile([128, tn], BF16, tag="xb16", bufs=3, name=f"xb16{blk}_{c}", padded_shape=[128, NB])
                nc.vector.tensor_copy(xb, xt)
                xsq = sb.tile([128, tn], BF16, tag="xsq", bufs=3, name=f"xsq{blk}_{c}", padded_shape=[128, NB])
                nc.scalar.activation(xsq, xt, AF.Square)
                nc.tensor.matmul(sum_ps, ones_k, xb,
                                 start=(c == 0), stop=(c == NC_T - 1))
                nc.tensor.matmul(sq_ps, ones_k, xsq,
                                 start=(c == 0), stop=(c == NC_T - 1))
                xs.append(xt)
            vmu = sb.tile([128, tn], F32, tag="vec", bufs=3, name=f"vmu{blk}", padded_shape=[128, NB])
            nc.vector.tensor_scalar_mul(vmu, sum_ps, 1.0 / C)
            # var = sq/C - mu^2
            vvar = sb.tile([128, tn], F32, tag="vec", bufs=3, name=f"vvar{blk}", padded_shape=[128, NB])
            nc.vector.scalar_tensor_tensor(vvar, vmu, -1.0, vmu, OP.mult, OP.mult)
            nc.vector.scalar_tensor_tensor(vvar, sq_ps, 1.0 / C, vvar, OP.mult, OP.add)
            vstd = sb.tile([128, tn], F32, tag="vec", bufs=3, name=f"vstd{blk}", padded_shape=[128, NB])
            nc.scalar.activation(vstd, vvar, AF.Sqrt, bias=eps_t)
            vrstd = sb.tile([128, tn], F32, tag="vec", bufs=3, name=f"vrstd{blk}", padded_shape=[128, NB])
            nc.vector.reciprocal_approx_fast(out=vrstd, in_=vstd)
            vcg = sb.tile([128, tn], F32, tag="bc", bufs=8, name=f"vcg{blk}", padded_shape=[128, NB])
            nc.sync.dma_start(vcg, cg_d[0:1, tsl].to_broadcast([128, tn]))
            if has_beta:
                vs = vrstd          # coef applied on the output instead
            else:
                vs = sb.tile([128, tn], F32, tag="bc", bufs=8, name=f"vs{blk}", padded_shape=[128, NB])
                nc.vector.tensor_mul(vs, vrstd, vcg)
            vb = sb.tile([128, tn], F32, tag="bc", bufs=8, name=f"vb{blk}", padded_shape=[128, NB])
            nc.vector.scalar_tensor_tensor(vb, vmu, -1.0, vs, OP.mult, OP.mult)
            return vs, vb, vcg, xs

        def normalize_phase(blk, vs, vb, xs):
            t0, tn = blocks[blk]
            xn = []
            for c in range(NC_T):
                xt = xs[c]
                nc.vector.tensor_mul(xt, xt, vs)
                xnc = sb.tile([128, tn], BF16, tag="xn", bufs=20, name=f"xn{blk}_{c}", padded_shape=[128, NB])
                nc.vector.tensor_add(xnc, xt, vb)
                xn.append(xnc)
            return xn

        def mm1_phase(blk, xn, mid_hook=None):
            t0, tn = blocks[blk]
            hid = []
            for h in range(NH_T):
                if h == 16 and mid_hook is not None:
                    mid_hook()
                w1t = sb.tile([128, C], BF16, tag="w1s", bufs=8, name=f"w1t{blk}_{h}")
                nc.scalar.dma_start(w1t, w1_d[h])
                pa = ps.tile([128, tn], F32, tag="mm", bufs=4, name=f"pa{blk}_{h}")
                for c in range(NC_T):
                    nc.tensor.matmul(pa, w1t[:, c * 128:(c + 1) * 128], xn[c],
                                     start=(c == 0), stop=(c == NC_T - 1))
                if has_beta:
                    nc.vector.tensor_scalar_add(pa, pa, b1sb[:, h:h + 1])
                # relu(x)^2 == max(x,0)*x; DVE may read only one PSUM operand
                rt = sb.tile([128, tn], BF16, tag="rt", bufs=3, name=f"r{blk}_{h}", padded_shape=[128, NB])
                nc.vector.tensor_scalar_max(rt, pa, 0.0)
                ht = sb.tile([128, tn], BF16, tag="hid", bufs=44, name=f"h{blk}_{h}", padded_shape=[128, NB])
                nc.vector.tensor_mul(ht, rt, pa)
                hid.append(ht)
            return hid

        def mm2_phase(blk, hid, vcf):
            t0, tn = blocks[blk]
            tsl = bass.ds(t0, tn)
            for c in range(NC_T):
                w2t = sb.tile([128, H], BF16, tag="w2s", bufs=4, name=f"w2t{blk}_{c}")
                nc.scalar.dma_start(w2t, w2_d[c])
                pb = ps.tile([128, tn], F32, tag="mm", bufs=4, name=f"pb{blk}_{c}")
                for h in range(NH_T):
                    nc.tensor.matmul(pb, w2t[:, h * 128:(h + 1) * 128], hid[h],
                                     start=(h == 0), stop=(h == NH_T - 1))
                ot = sb.tile([128, tn], F32, tag="out", bufs=4, name=f"o{blk}_{c}", padded_shape=[128, NB])
                if has_beta:
                    nc.vector.tensor_mul(ot, pb, vcf)
                else:
                    nc.vector.tensor_copy(ot, pb)
                nc.sync.dma_start(ygt_d[c * 128:(c + 1) * 128, tsl], ot)

        # Software pipeline: stats/normalize of blk+1 are emitted so the PE
        # runs them inside blk's mm1/mm2 stream with no gaps.
        vs0, vb0, vcf, xs0 = stats_phase(0)
        xn = normalize_phase(0, vs0, vb0, xs0)
        nxt = {}
        for blk in range(nblk):
            def mid_hook(b=blk):
                nxt.update(zip(("vs", "vb", "vcf", "xs"), stats_phase(b + 1)))
            hid = mm1_phase(blk, xn, mid_hook if blk + 1 < nblk else None)
            if blk + 1 < nblk:
                xn = normalize_phase(blk + 1, nxt["vs"], nxt["vb"], nxt["xs"])
            mm2_phase(blk, hid, vcf)
            if blk + 1 < nblk:
                vcf = nxt["vcf"]

    nc.compile()
    return nc


_KERNEL_CACHE = {}


def _get_kernel(NT: int, has_beta: bool):
    key = (NT, has_beta)
    if key not in _KERNEL_CACHE:
        _KERNEL_CACHE[key] = _build_kernel(NT, has_beta)
    return _KERNEL_CACHE[key]


def kernel(x, weights, gamma, beta, W1, W2, winners):
    x = np.asarray(x, dtype=np.float32)
    weights = np.asarray(weights, dtype=np.float32)
    gamma = np.asarray(gamma, dtype=np.float32)
    beta = np.asarray(beta, dtype=np.float32)
    W1 = np.asarray(W1, dtype=np.float32)
    W2 = np.asarray(W2, dtype=np.float32)
    winners = np.asarray(winners)

    B, T, C_ = x.shape
    E = W1.shape[0]
    assert C_ == C and E == N_CORES and W1.shape[2] == H

    x_flat = x.reshape(-1, C)
    win = winners.reshape(-1, 2)
    wts = weights.reshape(-1, 2)

    has_beta = bool(np.any(beta != 0.0))

    # ---- host-side routing (sharding prep) ----
    idxs, coefs = [], []
    for e in range(E):
        m = win == e
        tok = np.nonzero(m.any(axis=1))[0]
        cf = (wts * m).sum(axis=1)[tok]
        idxs.append(tok)
        coefs.append(cf.astype(np.float32))
    NT = int(np.ceil(max(len(t) for t in idxs) / 8) * 8)

    in_maps = []
    for e in range(E):
        tok, cf = idxs[e], coefs[e]
        n = len(tok)
        xg = np.zeros((NT, C), np.float32)
        xg[:n] = x_flat[tok]
        cg = np.zeros((1, NT), np.float32)
        # no beta: fold sqrt(coef) into the LN scale (relu^2 is 2-homogeneous
        # and W2 linear, so scaling xn by sqrt(c) scales the output by c).
        cg[0, :n] = cf if has_beta else np.sqrt(cf)
        w1g = (W1[e] * gamma[:, None]).astype(ml_dtypes.bfloat16)
        w1r = np.ascontiguousarray(
            w1g.reshape(NC_T, 128, NH_T, 128).transpose(2, 1, 0, 3)
        ).reshape(NH_T, 128, C)
        w2r = np.ascontiguousarray(
            W2[e].astype(ml_dtypes.bfloat16)
            .reshape(NH_T, 128, NC_T, 128).transpose(2, 1, 0, 3)
        ).reshape(NC_T, 128, H)
        m = {
            "xgt": np.ascontiguousarray(xg.T),
            "w1": w1r,
            "w2": w2r,
            "cg": cg,
        }
        if has_beta:
            b1 = (beta @ W1[e]).astype(np.float32)          # [H]
            m["bias1"] = np.ascontiguousarray(b1.reshape(NH_T, 128).T)
        in_maps.append(m)

    nc = _get_kernel(NT, has_beta)
    res = run_bass_kernel_spmd(nc, in_maps, list(range(N_CORES)))

    # ---- host-side unshard: scatter-add partial expert outputs ----
    out = x_flat.copy()
    for e in range(E):
        yg = res.results[e]["ygt"]                          # [C, NT]
        n = len(idxs[e])
        out[idxs[e]] += yg.T[:n]
    return out.reshape(B, T, C).astype(np.float32)
